# revision 1
# baseline (speedup 1.0000x reference)
"""Trainium2 Bass kernel for nn_Attention (B=4, C=256, L=2048, H=8 heads, D=64).

Sharding: head-parallel across 8 NeuronCores (1 head per core). Each core:
  - projects its head's Q/K/V from the full input x (channels-first, fp16),
  - runs attention in the S^T (keys-on-partitions) layout so softmax's
    denominator comes free from an appended ones-column in the V^T lhsT
    (M=65 matmul),
  - normalizes + casts its head output to fp16,
  - a per-batch AllToAll redistributes head outputs so each core owns all
    8 heads for l in [core*256, (core+1)*256) of every batch,
  - each core applies w_out + bias on its column shard.
Host reassembles the 8 column shards into the full [B, C, L] output.

All matmul operands are fp16 (PSUM accumulation is fp32); measured end-to-end
relative error vs the fp32 reference is ~6e-4, HW exec ~245us on 8 cores.
"""

import os
import sys

import numpy as np

sys.path.insert(0, "/opt/trn_rl_repo")

import concourse.bass as bass  # noqa: E402
import concourse.bacc as bacc  # noqa: E402
import concourse.tile as tile  # noqa: E402
import concourse.mybir as mybir  # noqa: E402
import concourse.bass_utils as bass_utils  # noqa: E402
from concourse.bass_interp import get_hw_module  # noqa: E402

B, C, L = 4, 256, 2048
H, D = 8, 64
NCORES = 8
N = B * L                # 8192 flattened (b, l) columns
LSH = L // NCORES        # 256 l-columns per core in the output shard
NBLK = 512               # matmul free-dim block
F32 = mybir.dt.float32
F16 = mybir.dt.float16
AF = mybir.ActivationFunctionType

_CACHE = {}


def _build():
    nc = bacc.Bacc("TRN2", target_bir_lowering=False, debug=False,
                   num_devices=NCORES)

    x_t = nc.dram_tensor("x_t", [2, 128, N], F16, kind="ExternalInput")
    # [c_lo, ch, (q|k) out] merged Q+K projection weights
    wqk_p = nc.dram_tensor("wqk_p", [128, 2, 128], F16, kind="ExternalInput")
    wv_p = nc.dram_tensor("wv_p", [128, 128], F16, kind="ExternalInput")
    wo_p = nc.dram_tensor("wo_p", [128, 4, 256], F16, kind="ExternalInput")
    bias2 = nc.dram_tensor("bias2", [128, 2], F32, kind="ExternalInput")
    out = nc.dram_tensor("out", [B, 2, 128, LSH], F32, kind="ExternalOutput")

    ident_d = nc.inline_tensor(np.eye(64, dtype=np.float16), name="ident64")

    with tile.TileContext(nc) as tc:
        with (
            tc.tile_pool(name="const", bufs=1) as cpool,
            tc.tile_pool(name="qk", bufs=2) as qkpool,
            tc.tile_pool(name="vt", bufs=2) as vtpool,
            tc.tile_pool(name="pt", bufs=3) as ptpool,
            tc.tile_pool(name="small", bufs=2) as spool,
            tc.tile_pool(name="gh", bufs=2) as ghpool,
            tc.tile_pool(name="psA", bufs=2, space="PSUM") as psA,
            tc.tile_pool(name="psO", bufs=2, space="PSUM") as psO,
            tc.tile_pool(name="psP", bufs=2, space="PSUM") as psP,
            tc.tile_pool(name="dram", bufs=1, space="DRAM") as dpool,
        ):
            # ---- constants / weights into SBUF (weights first: tiny and
            # needed by the first projection) ----
            wqk_sb = cpool.tile([128, 256], F16, name="wqk_sb")
            wv_sb = cpool.tile([128, 128], F16, name="wv_sb")
            wo_sb = cpool.tile([128, 1024], F16, name="wo_sb")
            bias_sb = cpool.tile([128, 2], F32, name="bias_sb")
            ident_sb = cpool.tile([64, 64], F16, name="ident_sb")
            nc.sync.dma_start(wqk_sb.rearrange("p (c o) -> p c o", c=2), wqk_p[:])
            nc.sync.dma_start(wv_sb[:], wv_p[:])
            nc.sync.dma_start(wo_sb.rearrange("p (c o) -> p c o", c=4), wo_p[:])
            nc.sync.dma_start(bias_sb[:], bias2[:])
            nc.sync.dma_start(ident_sb[:], ident_d[:])
            x_sb = cpool.tile([128, 2 * N], F16, name="x_sb")
            for s in range(8):          # n-chunk outer so early blocks land first
                for ch in range(2):
                    nc.sync.dma_start(
                        x_sb[:, ch * N + s * 1024:ch * N + (s + 1) * 1024],
                        x_t[ch, :, s * 1024:(s + 1) * 1024],
                    )

            bnc_in = [dpool.tile([NCORES, 64, LSH], F16, name=f"bnc_in{b}",
                                 tag=f"bnc_in{b}")
                      for b in range(B)]
            bnc_out = [dpool.tile([NCORES, 64, LSH], F16, name=f"bnc_out{b}",
                                  tag=f"bnc_out{b}")
                       for b in range(B)]

            qd = {}
            kd = {}
            vt3 = {}

            def emit_projvt(b, part):
                """Emit part (0..3) of batch b's QKV projection + V^T build."""
                if part == 0:
                    qd[b] = qkpool.tile([128, L], F16, name="qd", tag="qd")
                    kd[b] = qkpool.tile([128, L], F16, name="kd", tag="kd")
                if part < 2:
                    # Q+K merged projection: two n-blocks per part
                    for nb in (2 * part, 2 * part + 1):
                        ps = psP.tile([128, NBLK], F32, name="psqk", tag="psp")
                        for ch in range(2):
                            col0 = ch * N + b * L + nb * NBLK
                            nc.tensor.matmul(
                                ps[:], wqk_sb[:, ch * 128:(ch + 1) * 128],
                                x_sb[:, col0:col0 + NBLK],
                                start=(ch == 0), stop=(ch == 1))
                        nc.vector.tensor_copy(
                            qd[b][0:64, nb * NBLK:(nb + 1) * NBLK], ps[0:64, :])
                        nc.vector.tensor_copy(
                            kd[b][0:64, nb * NBLK:(nb + 1) * NBLK], ps[64:128, :])
                    if part == 1:  # duplicate into the upper partition halves
                        nc.vector.tensor_copy(qd[b][64:128, :], qd[b][0:64, :])
                        nc.vector.tensor_copy(kd[b][64:128, :], kd[b][0:64, :])
                    return
                if part == 2:
                    # V projection: col-strip packed pairs of n-blocks
                    vc = vtpool.tile([64, L], F16, name="vc", tag="vc")
                    vt3[b] = vtpool.tile([128, 16 * 65], F16, name="vt", tag="vt"
                                         ).rearrange("p (j e) -> p j e", e=65)
                    nc.vector.memset(vt3[b][:, :, 64], 1.0)
                    for nbp in range(2):
                        psv = psP.tile([128, NBLK], F32, name="psv", tag="psp")
                        for strip, nb in ((0, 2 * nbp), (64, 2 * nbp + 1)):
                            o_ap = psv[strip:strip + 64, :]
                            for ch in range(2):
                                col0 = ch * N + b * L + nb * NBLK
                                nc.tensor.matmul(
                                    o_ap, wv_sb[:, ch * 64:(ch + 1) * 64],
                                    x_sb[:, col0:col0 + NBLK],
                                    start=(ch == 0), stop=(ch == 1),
                                    tile_position=(0, strip))
                        nc.vector.tensor_copy(
                            vc[:, (2 * nbp) * NBLK:(2 * nbp + 1) * NBLK],
                            psv[0:64, :])
                        nc.vector.tensor_copy(
                            vc[:, (2 * nbp + 1) * NBLK:(2 * nbp + 2) * NBLK],
                            psv[64:128, :])
                    emit_projvt.vc = vc
                    return
                # part 3: V^T via PE transpose (+ ones column already memset)
                vc = emit_projvt.vc
                for jt in range(8):
                    pst = psP.tile([128, 128], F16, name="pst", tag="psp")
                    nc.tensor.transpose(
                        pst[:, 0:64],
                        vc[:, (2 * jt) * 128:(2 * jt + 1) * 128], ident_sb[:])
                    nc.tensor.transpose(
                        pst[:, 64:128],
                        vc[:, (2 * jt + 1) * 128:(2 * jt + 2) * 128], ident_sb[:])
                    nc.vector.tensor_copy(
                        vt3[b][:, 2 * jt:2 * jt + 2, 0:64],
                        pst.rearrange("p (j e) -> p j e", e=64))

            def emit_attention_iblk(b, ib):
                pso = psO.tile([65, NBLK], F32, name="pso", tag="pso")
                for jp in range(8):
                    jA, jB = 2 * jp, 2 * jp + 1
                    pss = psA.tile([128, 2 * NBLK], F32, name="pss", tag="pss")
                    nc.tensor.matmul(
                        pss[:, 0:NBLK],
                        kd[b][0:64, jA * 128:(jA + 1) * 128],
                        qd[b][0:64, ib * NBLK:(ib + 1) * NBLK],
                        start=True, stop=True, tile_position=(0, 0))
                    nc.tensor.matmul(
                        pss[:, NBLK:2 * NBLK],
                        kd[b][64:128, jB * 128:(jB + 1) * 128],
                        qd[b][64:128, ib * NBLK:(ib + 1) * NBLK],
                        start=True, stop=True, tile_position=(64, 0))
                    pt = ptpool.tile([128, 2 * NBLK], F16, name="pt", tag="pt")
                    nc.scalar.activation(pt[:], pss[:], AF.Exp)
                    nc.tensor.matmul(
                        pso[:], vt3[b][:, jA, :], pt[:, 0:NBLK],
                        start=(jp == 0), stop=False)
                    nc.tensor.matmul(
                        pso[:], vt3[b][:, jB, :], pt[:, NBLK:2 * NBLK],
                        start=False, stop=(jp == 7))
                recip = spool.tile([1, NBLK], F32, name="recip", tag="recip")
                nc.vector.reciprocal(recip[:], pso[64:65, :])
                bc = spool.tile([64, NBLK], F32, name="bc", tag="bc")
                nc.gpsimd.partition_broadcast(bc[:], recip[:])
                on = spool.tile([64, NBLK], F16, name="on", tag="on")
                nc.vector.tensor_mul(on[:], pso[0:64, :], bc[:])
                # split the 512 columns into the two destination l-shards
                for half in range(2):
                    sh = 2 * ib + half
                    nc.sync.dma_start(bnc_in[b][sh, :, :],
                                      on[:, half * LSH:(half + 1) * LSH])

            def emit_a2a(b):
                nc.gpsimd.collective_compute(
                    "AllToAll", mybir.AluOpType.bypass,
                    replica_groups=[list(range(NCORES))],
                    ins=[bnc_in[b].opt()], outs=[bnc_out[b].opt()])

            def emit_yproj(b):
                """Per-batch gather + output projection (after collective)."""
                gh = ghpool.tile([128, 4 * LSH], F16, name="gh", tag="gh")
                for hc in range(4):
                    for hp in range(2):
                        nc.sync.dma_start(
                            gh[hp * 64:(hp + 1) * 64, hc * LSH:(hc + 1) * LSH],
                            bnc_out[b][hc * 2 + hp, :, :])
                for oh in range(2):
                    psy = psP.tile([128, LSH], F32, name="psy", tag="psp")
                    for c in range(4):
                        nc.tensor.matmul(
                            psy[:],
                            wo_sb[:, c * 256 + oh * 128:c * 256 + (oh + 1) * 128],
                            gh[:, c * LSH:(c + 1) * LSH],
                            start=(c == 0), stop=(c == 3))
                    y = spool.tile([128, LSH], F32, name="y", tag="y")
                    nc.vector.tensor_scalar_add(y[:], psy[:], bias_sb[:, oh:oh + 1])
                    nc.sync.dma_start(out[b, oh, :, :], y[:])

            for part in range(4):
                emit_projvt(0, part)
            for b in range(B):
                for ib in range(4):
                    emit_attention_iblk(b, ib)
                    if b + 1 < B:
                        emit_projvt(b + 1, ib)
                emit_a2a(b)
            for b in range(B):
                emit_yproj(b)

    nc.compile()
    nc.m = get_hw_module(nc.m)
    return nc


def _prep_in_maps(x, w_qkv, w_out, b_out):
    scale = float(D) ** -0.5
    x = np.asarray(x, np.float32)
    w_qkv = np.asarray(w_qkv, np.float32)
    w_out = np.asarray(w_out, np.float32)
    b_out = np.asarray(b_out, np.float32)

    x_in = np.ascontiguousarray(
        x.transpose(1, 0, 2).reshape(C, N).reshape(2, 128, N)).astype(np.float16)
    wq = w_qkv[0:512].reshape(H, D, C) * scale
    wk = w_qkv[512:1024].reshape(H, D, C)
    wv = w_qkv[1024:1536].reshape(H, D, C)

    wo_p = np.ascontiguousarray(
        w_out.T.reshape(4, 2, 64, 256).transpose(1, 2, 0, 3).reshape(128, 4, 256)
    ).astype(np.float16)
    bias2 = np.ascontiguousarray(b_out.reshape(2, 128).T)

    in_maps = []
    for h in range(NCORES):
        # [c, 128] per half with columns [q 64 | k 64] stacked -> M=128
        wqk = np.concatenate([wq[h].T, wk[h].T], axis=1)  # [256, 128]
        wqk_packed = np.ascontiguousarray(
            wqk.reshape(2, 128, 128).transpose(1, 0, 2)).astype(np.float16)
        wv_packed = np.ascontiguousarray(
            wv[h].T.reshape(2, 128, 64).transpose(1, 0, 2).reshape(128, 128)
        ).astype(np.float16)
        in_maps.append({
            "x_t": x_in,
            "wqk_p": wqk_packed,
            "wv_p": wv_packed,
            "wo_p": wo_p,
            "bias2": bias2,
        })
    return in_maps


def _run(inputs, trace=False):
    if "nc" not in _CACHE:
        _CACHE["nc"] = _build()
    nc = _CACHE["nc"]
    in_maps = _prep_in_maps(**inputs)
    res = bass_utils.run_bass_kernel_spmd(
        nc, in_maps, core_ids=list(range(NCORES)), trace=trace)
    y = np.empty((B, C, L), np.float32)
    for j in range(NCORES):
        shard = res.results[j]["out"].reshape(B, C, LSH)
        y[:, :, j * LSH:(j + 1) * LSH] = shard
    return y, res


def kernel(x, w_qkv, w_out, b_out):
    y, _ = _run(dict(x=x, w_qkv=w_qkv, w_out=w_out, b_out=b_out), trace=False)
    return y



# revision 7
# speedup vs baseline: 1.0234x; 1.0234x over previous
"""Trainium2 Bass kernel for nn_Attention (B=4, C=256, L=2048, H=8 heads, D=64).

Sharding: head-parallel across 8 NeuronCores (1 head per core). Each core:
  - projects its head's Q/K/V from the full input x (channels-first, fp16),
  - runs attention in the S^T (keys-on-partitions) layout so softmax's
    denominator comes free from an appended ones-column in the V^T lhsT
    (M=65 matmul),
  - normalizes + casts its head output to fp16,
  - a per-batch AllToAll redistributes head outputs so each core owns all
    8 heads for l in [core*256, (core+1)*256) of every batch,
  - each core applies w_out + bias on its column shard.
Host reassembles the 8 column shards into the full [B, C, L] output.

All matmul operands are fp16 (PSUM accumulation is fp32); measured end-to-end
relative error vs the fp32 reference is ~6e-4, HW exec ~245us on 8 cores.
"""

import os
import sys

import numpy as np

sys.path.insert(0, "/opt/trn_rl_repo")

import concourse.bass as bass  # noqa: E402
import concourse.bacc as bacc  # noqa: E402
import concourse.tile as tile  # noqa: E402
import concourse.mybir as mybir  # noqa: E402
import concourse.bass_utils as bass_utils  # noqa: E402
from concourse.bass_interp import get_hw_module  # noqa: E402

B, C, L = 4, 256, 2048
H, D = 8, 64
NCORES = 8
N = B * L                # 8192 flattened (b, l) columns
LSH = L // NCORES        # 256 l-columns per core in the output shard
NBLK = 512               # matmul free-dim block
F32 = mybir.dt.float32
F16 = mybir.dt.float16
AF = mybir.ActivationFunctionType

_CACHE = {}


def _build():
    nc = bacc.Bacc("TRN2", target_bir_lowering=False, debug=False,
                   num_devices=NCORES)

    x_t = nc.dram_tensor("x_t", [2, 128, N], F16, kind="ExternalInput")
    # [c_lo, ch, (q|k) out] merged Q+K projection weights
    wqk_p = nc.dram_tensor("wqk_p", [128, 2, 128], F16, kind="ExternalInput")
    wv_p = nc.dram_tensor("wv_p", [128, 128], F16, kind="ExternalInput")
    wo_p = nc.dram_tensor("wo_p", [128, 4, 256], F16, kind="ExternalInput")
    bias2 = nc.dram_tensor("bias2", [128, 2], F32, kind="ExternalInput")
    out = nc.dram_tensor("out", [B, 2, 128, LSH], F32, kind="ExternalOutput")

    with tile.TileContext(nc) as tc:
        with (
            tc.tile_pool(name="const", bufs=1) as cpool,
            tc.tile_pool(name="qk", bufs=2) as qkpool,
            tc.tile_pool(name="vt", bufs=2) as vtpool,
            tc.tile_pool(name="pt", bufs=4) as ptpool,
            tc.tile_pool(name="small", bufs=4) as spool,
            tc.tile_pool(name="gh", bufs=2) as ghpool,
            tc.tile_pool(name="psA", bufs=2, space="PSUM") as psA,
            tc.tile_pool(name="psO", bufs=2, space="PSUM") as psO,
            tc.tile_pool(name="psP", bufs=2, space="PSUM") as psP,
            tc.tile_pool(name="dram", bufs=1, space="DRAM") as dpool,
        ):
            # ---- constants / weights into SBUF (weights first: tiny and
            # needed by the first projection; batch-0 x chunks next so the
            # first matmuls can start ASAP; wo/bias are only needed ~90us in) ----
            wqk_sb = cpool.tile([128, 256], F16, name="wqk_sb")
            wv_sb = cpool.tile([128, 128], F16, name="wv_sb")
            wo_sb = cpool.tile([128, 1024], F16, name="wo_sb")
            bias_sb = cpool.tile([128, 2], F32, name="bias_sb")
            nc.sync.dma_start(wqk_sb.rearrange("p (c o) -> p c o", c=2), wqk_p[:])
            nc.sync.dma_start(wv_sb[:], wv_p[:])
            x_sb = cpool.tile([128, 2 * N], F16, name="x_sb")
            for s in range(2):          # batch-0 columns first
                for ch in range(2):
                    nc.sync.dma_start(
                        x_sb[:, ch * N + s * 1024:ch * N + (s + 1) * 1024],
                        x_t[ch, :, s * 1024:(s + 1) * 1024],
                    )
            nc.sync.dma_start(wo_sb.rearrange("p (c o) -> p c o", c=4), wo_p[:])
            nc.sync.dma_start(bias_sb[:], bias2[:])
            for s in range(2, 8):
                for ch in range(2):
                    nc.sync.dma_start(
                        x_sb[:, ch * N + s * 1024:ch * N + (s + 1) * 1024],
                        x_t[ch, :, s * 1024:(s + 1) * 1024],
                    )

            bnc_in = [dpool.tile([NCORES, 64, LSH], F16, name=f"bnc_in{b}",
                                 tag=f"bnc_in{b}")
                      for b in range(B)]
            bnc_out = [dpool.tile([NCORES, 64, LSH], F16, name=f"bnc_out{b}",
                                  tag=f"bnc_out{b}")
                       for b in range(B)]

            qd = {}
            kd = {}
            vt3 = {}

            def emit_projvt(b, part):
                """Emit part (0..3) of batch b's QKV projection + V^T build."""
                if part == 0:
                    qd[b] = qkpool.tile([128, L], F16, name="qd", tag="qd")
                    kd[b] = qkpool.tile([128, L], F16, name="kd", tag="kd")
                if part < 2:
                    # Q+K merged projection: two n-blocks per part
                    for nb in (2 * part, 2 * part + 1):
                        ps = psP.tile([128, NBLK], F32, name="psqk", tag="psp")
                        for ch in range(2):
                            col0 = ch * N + b * L + nb * NBLK
                            nc.tensor.matmul(
                                ps[:], wqk_sb[:, ch * 128:(ch + 1) * 128],
                                x_sb[:, col0:col0 + NBLK],
                                start=(ch == 0), stop=(ch == 1))
                        nc.vector.tensor_copy(
                            qd[b][0:64, nb * NBLK:(nb + 1) * NBLK], ps[0:64, :])
                        nc.vector.tensor_copy(
                            kd[b][0:64, nb * NBLK:(nb + 1) * NBLK], ps[64:128, :])
                    if part == 1:  # duplicate into the upper partition halves
                        nc.vector.tensor_copy(qd[b][64:128, :], qd[b][0:64, :])
                        nc.vector.tensor_copy(kd[b][64:128, :], kd[b][0:64, :])
                    return
                # parts 2/3: V^T computed directly with x as the stationary
                # operand — out[l, d] = sum_c x[c, l] * wv[c, d] — no PE
                # transposes, no intermediate V tile.
                if part == 2:
                    vt3[b] = vtpool.tile([128, 16 * 65], F16, name="vt", tag="vt"
                                         ).rearrange("p (j e) -> p j e", e=65)
                    nc.vector.memset(vt3[b][:, :, 64], 1.0)
                for jp in range(4 * (part - 2), 4 * (part - 1)):
                    pst = psP.tile([128, 128], F32, name="pst", tag="psp")
                    for half in range(2):
                        jt = 2 * jp + half
                        for ch in range(2):
                            col0 = ch * N + b * L + jt * 128
                            nc.tensor.matmul(
                                pst[:, half * 64:(half + 1) * 64],
                                x_sb[:, col0:col0 + 128],
                                wv_sb[:, ch * 64:(ch + 1) * 64],
                                start=(ch == 0), stop=(ch == 1))
                    nc.vector.tensor_copy(
                        vt3[b][:, 2 * jp:2 * jp + 2, 0:64],
                        pst.rearrange("p (j e) -> p j e", e=64))

            def emit_attention_iblk(b, ib, post=None):
                """Software-pipelined S/Exp/PV: PV(jp-1) is emitted after
                S(jp) so the PE streams S(jp) while the ACT engine runs
                Exp(jp-1); the PE then finds PV(jp-1) ready and never waits a
                full Exp latency inside the loop. `post` (if given) is called
                mid-loop to slip independent work (yproj of a previous batch)
                into the engine streams."""
                pso = psO.tile([65, NBLK], F32, name="pso", tag="pso")
                pts = {}

                def emit_s(jp):
                    jA, jB = 2 * jp, 2 * jp + 1
                    pss = psA.tile([128, 2 * NBLK], F32, name="pss", tag="pss")
                    nc.tensor.matmul(
                        pss[:, 0:NBLK],
                        kd[b][0:64, jA * 128:(jA + 1) * 128],
                        qd[b][0:64, ib * NBLK:(ib + 1) * NBLK],
                        start=True, stop=True, tile_position=(0, 0))
                    nc.tensor.matmul(
                        pss[:, NBLK:2 * NBLK],
                        kd[b][64:128, jB * 128:(jB + 1) * 128],
                        qd[b][64:128, ib * NBLK:(ib + 1) * NBLK],
                        start=True, stop=True, tile_position=(64, 0))
                    pt = ptpool.tile([128, 2 * NBLK], F16, name="pt", tag="pt")
                    nc.scalar.activation(pt[:], pss[:], AF.Exp)
                    pts[jp] = pt

                def emit_pv(jp):
                    jA, jB = 2 * jp, 2 * jp + 1
                    pt = pts.pop(jp)
                    nc.tensor.matmul(
                        pso[:], vt3[b][:, jA, :], pt[:, 0:NBLK],
                        start=(jp == 0), stop=False)
                    nc.tensor.matmul(
                        pso[:], vt3[b][:, jB, :], pt[:, NBLK:2 * NBLK],
                        start=False, stop=(jp == 7))

                emit_s(0)
                for jp in range(1, 8):
                    emit_s(jp)
                    emit_pv(jp - 1)
                    if jp == 4 and post is not None:
                        post()
                emit_pv(7)
                recip = spool.tile([1, NBLK], F32, name="recip", tag="recip")
                nc.vector.reciprocal(recip[:], pso[64:65, :])
                bc = spool.tile([64, NBLK], F32, name="bc", tag="bc")
                nc.gpsimd.partition_broadcast(bc[:], recip[:])
                on = spool.tile([64, NBLK], F16, name="on", tag="on")
                nc.vector.tensor_mul(on[:], pso[0:64, :], bc[:])
                # split the 512 columns into the two destination l-shards
                for half in range(2):
                    sh = 2 * ib + half
                    nc.sync.dma_start(bnc_in[b][sh, :, :],
                                      on[:, half * LSH:(half + 1) * LSH])

            def emit_a2a(b):
                nc.gpsimd.collective_compute(
                    "AllToAll", mybir.AluOpType.bypass,
                    replica_groups=[list(range(NCORES))],
                    ins=[bnc_in[b].opt()], outs=[bnc_out[b].opt()])

            def emit_yproj(b):
                """Per-batch gather + output projection (after collective)."""
                gh = ghpool.tile([128, 4 * LSH], F16, name="gh", tag="gh")
                for hc in range(4):
                    for hp in range(2):
                        nc.sync.dma_start(
                            gh[hp * 64:(hp + 1) * 64, hc * LSH:(hc + 1) * LSH],
                            bnc_out[b][hc * 2 + hp, :, :])
                for oh in range(2):
                    psy = psP.tile([128, LSH], F32, name="psy", tag="psp")
                    for c in range(4):
                        nc.tensor.matmul(
                            psy[:],
                            wo_sb[:, c * 256 + oh * 128:c * 256 + (oh + 1) * 128],
                            gh[:, c * LSH:(c + 1) * LSH],
                            start=(c == 0), stop=(c == 3))
                    y = spool.tile([128, LSH], F32, name="y", tag="y")
                    nc.vector.tensor_scalar_add(y[:], psy[:], bias_sb[:, oh:oh + 1])
                    nc.sync.dma_start(out[b, oh, :, :], y[:])

            # yproj(b-1) is slipped into the middle of batch b's attention
            # (by then a2a(b-1) has completed) so only a2a(3)+yproj(3) sit on
            # the tail of the critical path.
            for part in range(4):
                emit_projvt(0, part)
            for b in range(B):
                for ib in range(4):
                    post = None
                    if ib == 3 and b > 0:
                        post = (lambda bb: lambda: emit_yproj(bb))(b - 1)
                    emit_attention_iblk(b, ib, post=post)
                    if b + 1 < B:
                        emit_projvt(b + 1, ib)
                emit_a2a(b)
            emit_yproj(B - 1)

    nc.compile()
    nc.m = get_hw_module(nc.m)
    return nc


def _prep_in_maps(x, w_qkv, w_out, b_out):
    scale = float(D) ** -0.5
    x = np.asarray(x, np.float32)
    w_qkv = np.asarray(w_qkv, np.float32)
    w_out = np.asarray(w_out, np.float32)
    b_out = np.asarray(b_out, np.float32)

    x_in = np.ascontiguousarray(
        x.transpose(1, 0, 2).reshape(C, N).reshape(2, 128, N)).astype(np.float16)
    wq = w_qkv[0:512].reshape(H, D, C) * scale
    wk = w_qkv[512:1024].reshape(H, D, C)
    wv = w_qkv[1024:1536].reshape(H, D, C)

    wo_p = np.ascontiguousarray(
        w_out.T.reshape(4, 2, 64, 256).transpose(1, 2, 0, 3).reshape(128, 4, 256)
    ).astype(np.float16)
    bias2 = np.ascontiguousarray(b_out.reshape(2, 128).T)

    in_maps = []
    for h in range(NCORES):
        # [c, 128] per half with columns [q 64 | k 64] stacked -> M=128
        wqk = np.concatenate([wq[h].T, wk[h].T], axis=1)  # [256, 128]
        wqk_packed = np.ascontiguousarray(
            wqk.reshape(2, 128, 128).transpose(1, 0, 2)).astype(np.float16)
        wv_packed = np.ascontiguousarray(
            wv[h].T.reshape(2, 128, 64).transpose(1, 0, 2).reshape(128, 128)
        ).astype(np.float16)
        in_maps.append({
            "x_t": x_in,
            "wqk_p": wqk_packed,
            "wv_p": wv_packed,
            "wo_p": wo_p,
            "bias2": bias2,
        })
    return in_maps


def _run(inputs, trace=False):
    if "nc" not in _CACHE:
        _CACHE["nc"] = _build()
    nc = _CACHE["nc"]
    in_maps = _prep_in_maps(**inputs)
    res = bass_utils.run_bass_kernel_spmd(
        nc, in_maps, core_ids=list(range(NCORES)), trace=trace)
    y = np.empty((B, C, L), np.float32)
    for j in range(NCORES):
        shard = res.results[j]["out"].reshape(B, C, LSH)
        y[:, :, j * LSH:(j + 1) * LSH] = shard
    return y, res


def kernel(x, w_qkv, w_out, b_out):
    y, _ = _run(dict(x=x, w_qkv=w_qkv, w_out=w_out, b_out=b_out), trace=False)
    return y

